# revision 6
# baseline (speedup 1.0000x reference)
"""AdditiveAttention on 8 TRN2 NeuronCores — harmonic-doubling edition.

Math: out = softmax_k(mask(sum_h w_v[h] * tanh(qp[b,q,h] + kp[b,k,h]))) @ values
with qp = queries @ W_q^T, kp = keys @ W_k^T, mask from valid_lens (B,).

tanh(u) ~= sum_{l=0..3} b_l sin(2^l * w0 * u): four harmonics in a pure
power-of-two ladder, so every level comes from the previous by one doubling:
    s[l+1] = s[l]*c[l]   (stored scaled by 1/2^(l+1))
    c[l+1] = 1 - 2*4^l*s[l]^2
No odd-harmonic Chebyshev chain at all. sin(2^l w0 (q+k)) factorizes by angle
addition into PE matmuls contracting over (h, level, trig).

Engine split: ACT does base sin/cos (args kept in [-pi,pi] via w0 choice) plus
softmax exp; DVE does the doubling products (tensor_tensor, 2x fp16) and the
b_l*w_v scale columns (tensor_scalar with per-partition AP scalars, 4x);
Pool (gpsimd) issues one input DMA and takes non-critical scale work; PE does
projections, score matmuls, transposes, AV.

Last-level trick: the k-side cos(8 w0 x) enters only as a matmul operand, so
it is replaced by ct = 1 - cos = 2 sin^2(4 w0 x), computed in ONE ACT Square
op straight from level-2 sin; the dropped constant is a per-query-row score
offset, invisible to softmax. The sign flip folds into the host-built scale
column.

Masking: keys truncated/padded to KP (multiple of 128) >= max(valid_lens); a
rank-1 matmul row adds -60000 to padded score columns so exp underflows to 0.

Sharding: core c handles batch c//2, query rows (c%2)*256..+256.
w0 and b_l are fit host-side from the actual inputs at call time.
"""

import math
from contextlib import ExitStack

import numpy as np

import concourse.bass as bass
import concourse.mybir as mybir
import concourse.tile as tile
from concourse import bacc
from concourse.bass_utils import run_bass_kernel_spmd

B, Q, K, D, H, V = 4, 512, 512, 256, 256, 256
NCORES = 8
NQ = (B * Q) // NCORES          # 256 query rows per core
NLEV = 4                        # harmonics 2^l * w0, l = 0..3
NEGM = -60000.0
FP32 = mybir.dt.float32
FP16 = mybir.dt.float16
AX = mybir.AxisListType
ALU = mybir.AluOpType
ACTF = mybir.ActivationFunctionType


def fit_series(qp, kp, vls):
    """Range analysis + weighted least-squares fit of tanh on the power-of-2
    harmonic ladder. qp/kp: [b][h, *]."""
    umax, xmax = 0.0, 0.0
    for b in range(B):
        kv = kp[b][:, : vls[b]]
        umax = max(umax, (qp[b].max(1) + kv.max(1)).max(),
                   -(qp[b].min(1) + kv.min(1)).min())
        xmax = max(xmax, np.abs(qp[b]).max(), np.abs(kv).max())
    P = max(2.0 * (umax + 0.15), 4.0 * xmax + 0.08)
    w0 = 2.0 * np.pi / P
    u = np.linspace(-(umax + 0.05), umax + 0.05, 4001)
    A = np.stack([np.sin((2.0 ** l) * w0 * u) for l in range(NLEV)], 1)
    wgt = np.exp(-(u ** 2) / (2 * 1.4 ** 2)) + 1e-3
    sw = np.sqrt(wgt)[:, None]
    bco, *_ = np.linalg.lstsq(A * sw, np.tanh(u) * sw[:, 0], rcond=None)
    return float(w0), bco.astype(np.float64)


def pack_layout(KP):
    """Column offsets inside the packed (128, PX) fp16 input tensor. Order
    matters: wk|kT first (k-side spine starts first), wq|qT second, v|ident
    last (needed late)."""
    NK = KP // 128
    names = ([("wk0", H), ("wk1", H), ("kT0", KP), ("kT1", KP),
              ("wq0", H), ("wq1", H), ("qT0", NQ), ("qT1", NQ)]
             + [(f"v{i}", V) for i in range(NK)] + [("ident", 128)])
    off, x = {}, 0
    for nm, w in names:
        off[nm] = x
        x += w
    cutA = off["wq0"]            # end of k-side chunk
    cutB = off["v0"]             # end of q-side chunk
    return off, x, (cutA, cutB)


def build_nc(w0, bco, KP):
    NK = KP // 128
    QW = 2 * NQ                  # q-side width (2 h-chunks)
    KW = 2 * KP                  # k-side width
    CW = QW + KW
    OFF, PX, (CUTA, CUTB) = pack_layout(KP)
    HPI = math.pi / 2
    NCOL = 2 * (NLEV + 1)        # per hc: col_0..2, colS_3, colC_3

    nc = bacc.Bacc()
    pack = nc.declare_dram_parameter("pack", [128, PX], FP16, isOutput=False)
    mo = nc.declare_dram_parameter("mo", [1, KP + 128], FP16, isOutput=False)
    cols = nc.declare_dram_parameter("cols", [128, NCOL], FP32, isOutput=False)
    out_d = nc.declare_dram_parameter("out", [NQ, V], FP32, isOutput=True)

    with TileCtx(nc) as (tc, ctx):
        inp = ctx.enter_context(tc.tile_pool(name="inp", bufs=1))
        harm = ctx.enter_context(tc.tile_pool(name="harm", bufs=1))
        qbp = ctx.enter_context(tc.tile_pool(name="qb", bufs=1))
        sm = ctx.enter_context(tc.tile_pool(name="sm", bufs=1))
        ps_pr = ctx.enter_context(tc.tile_pool(name="psP", bufs=1, space="PSUM"))
        ps_sc = ctx.enter_context(tc.tile_pool(name="psS", bufs=1, space="PSUM"))
        ps_pt = ctx.enter_context(tc.tile_pool(name="psT", bufs=1, space="PSUM"))

        # ---- tiny host-free init on Pool: warmup tile + pi/2 bias column
        warm = inp.tile([1, 128], FP16, tag="warm", name="warm")
        nc.gpsimd.memset(warm, 0.25)
        hpi = inp.tile([128, 1], FP32, tag="hpi", name="hpi")
        nc.gpsimd.memset(hpi, HPI)

        # ---- input DMAs on three parallel queues (HWDGE: SP/ACT; SWDGE: Pool)
        big = inp.tile([128, PX], FP16, tag="big", name="big")
        nc.gpsimd.dma_start(out=big[:, :CUTA], in_=pack[:, :CUTA])       # wk|kT
        nc.scalar.dma_start(out=big[:, CUTA:CUTB], in_=pack[:, CUTA:CUTB])  # wq|qT
        mo_sb = inp.tile([1, KP + 128], FP16, tag="mo", name="mo_sb")
        cols_sb = inp.tile([128, NCOL], FP32, tag="cols", name="cols_sb")
        nc.sync.dma_start(out=mo_sb, in_=mo[:, :])
        nc.sync.dma_start(out=cols_sb, in_=cols[:, :])
        nc.sync.dma_start(out=big[:, CUTB:], in_=pack[:, CUTB:])         # v|ident

        # ACT: trigger the Sin table load ASAP via a warmup op
        nc.scalar.activation(warm, warm, ACTF.Sin, scale=0.001)

        wk_sb = [big[:, OFF[f"wk{i}"]: OFF[f"wk{i}"] + H] for i in range(2)]
        kT_sb = [big[:, OFF[f"kT{i}"]: OFF[f"kT{i}"] + KP] for i in range(2)]
        wq_sb = [big[:, OFF[f"wq{i}"]: OFF[f"wq{i}"] + H] for i in range(2)]
        qT_sb = [big[:, OFF[f"qT{i}"]: OFF[f"qT{i}"] + NQ] for i in range(2)]
        v_sb = [big[:, OFF[f"v{i}"]: OFF[f"v{i}"] + V] for i in range(NK)]
        mrow_sb = mo_sb[:, :KP]
        ones_r = mo_sb[:, KP: KP + 128]
        ident = big[:, OFF["ident"]: OFF["ident"] + 128]

        def colAP(hc, j):
            return cols_sb[:, hc * (NLEV + 1) + j: hc * (NLEV + 1) + j + 1]

        # ---- projections: kp first (k spine), then qp
        kp_ps = [ps_pr.tile([128, KP], FP32, tag=f"kp{hc}", name=f"kp{hc}")
                 for hc in range(2)]
        for hc in range(2):
            for dc in range(2):
                nc.tensor.matmul(kp_ps[hc], wk_sb[dc][:, 128 * hc: 128 * (hc + 1)],
                                 kT_sb[dc], start=(dc == 0), stop=(dc == 1))
        qp_ps = [ps_pr.tile([128, NQ], FP32, tag=f"qp{hc}", name=f"qp{hc}")
                 for hc in range(2)]
        for hc in range(2):
            for dc in range(2):
                nc.tensor.matmul(qp_ps[hc], wq_sb[dc][:, 128 * hc: 128 * (hc + 1)],
                                 qT_sb[dc], start=(dc == 0), stop=(dc == 1))

        # ---- masks open the score accumulation groups early
        sc_ps = [ps_sc.tile([128, KP], FP32, tag=f"sc{qt}", name=f"sc{qt}")
                 for qt in range(2)]
        for qt in range(2):
            nc.tensor.matmul(sc_ps[qt], ones_r, mrow_sb, start=True, stop=False)

        # ---- harmonic tiles: T[l] = [s | c] over [q-side | k-side]
        # layout [128, 2, CW]: [:, 0] = s (stored sin/2^l), [:, 1] = c
        T = [harm.tile([128, 2, CW], FP16, tag=f"T{l}", name=f"T{l}")
             for l in range(NLEV)]
        s = [T[l][:, 0] for l in range(NLEV)]
        c = [T[l][:, 1] for l in range(NLEV)]
        sq = [harm.tile([128, CW], FP16, tag=f"sq{l}", name=f"sq{l}")
              for l in range(2)]
        sq2q = harm.tile([128, QW], FP16, tag="sq2q", name="sq2q")
        ct3k = harm.tile([128, KW], FP16, tag="ct3k", name="ct3k")

        def qsl(ap):                 # q-side slice of a [128, CW] view
            return ap[:, :QW]

        def ksl(ap):                 # k-side slice
            return ap[:, QW:]

        # base level 0: k-side per hc (earliest start), then q-side
        for hc in range(2):
            sl = slice(QW + hc * KP, QW + (hc + 1) * KP)
            nc.scalar.activation(s[0][:, sl], kp_ps[hc], ACTF.Sin, scale=w0)
        for hc in range(2):
            sl = slice(QW + hc * KP, QW + (hc + 1) * KP)
            nc.scalar.activation(c[0][:, sl], kp_ps[hc], ACTF.Sin, scale=w0,
                                 bias=hpi)
        for hc in range(2):
            sl = slice(hc * NQ, (hc + 1) * NQ)
            nc.scalar.activation(s[0][:, sl], qp_ps[hc], ACTF.Sin, scale=w0)
        for hc in range(2):
            sl = slice(hc * NQ, (hc + 1) * NQ)
            nc.scalar.activation(c[0][:, sl], qp_ps[hc], ACTF.Sin, scale=w0,
                                 bias=hpi)

        # ---- scaled q-side stationaries SCb[l] = [Sqb | Cqb], [128, 2, QW]
        SCb = [qbp.tile([128, 2, QW], FP16, tag=f"SCb{l}", name=f"SCb{l}")
               for l in range(NLEV)]

        def scale_level(l, eng):
            """SCb[l] = col_l * [s_l | c_l]|q in one tensor_scalar per hc pair
            of columns (single col covers both trig; see module docstring)."""
            for hc in range(2):
                qs = slice(hc * NQ, (hc + 1) * NQ)
                eng.tensor_scalar(SCb[l][:, :, qs], T[l][:, :, qs],
                                  colAP(hc, l), None, ALU.mult)

        # ---- doubling chain, interleaved with scales and score matmuls.
        # transitions l -> l+1:  sq_l = s_l^2 ;  c_{l+1} = 1 - 2*4^l*sq_l ;
        #                        s_{l+1} = s_l * c_l
        score_mm = []                # collect (l, qt, stationary, moving) order

        def emit_scores(l, qt, last=False):
            """8 matmuls accumulate level l into sc_ps[qt]."""
            for hc in range(2):
                q128 = slice(hc * NQ + qt * 128, hc * NQ + (qt + 1) * 128)
                k_sl = slice(QW + hc * KP, QW + (hc + 1) * KP)
                if l < 3:
                    mvS, mvC = s[l][:, k_sl], c[l][:, k_sl]
                else:
                    mvS, mvC = s[3][:, k_sl], ct3k[:, hc * KP:(hc + 1) * KP]
                fin = last and hc == 1
                nc.tensor.matmul(sc_ps[qt], SCb[l][:, 0, q128], mvC,
                                 start=False, stop=False)
                nc.tensor.matmul(sc_ps[qt], SCb[l][:, 1, q128], mvS,
                                 start=False, stop=fin)

        # level 0 scale + scores
        scale_level(0, nc.vector)
        emit_scores(0, 0)
        emit_scores(0, 1)

        # transition 0->1
        nc.vector.tensor_mul(sq[0], s[0], s[0])
        nc.vector.tensor_scalar(c[1], sq[0], -2.0, 1.0, ALU.mult, ALU.add)
        nc.vector.tensor_mul(s[1], s[0], c[0])
        scale_level(1, nc.gpsimd)
        emit_scores(1, 0)
        emit_scores(1, 1)

        # transition 1->2
        nc.vector.tensor_mul(sq[1], s[1], s[1])
        nc.vector.tensor_scalar(c[2], sq[1], -8.0, 1.0, ALU.mult, ALU.add)
        nc.vector.tensor_mul(s[2], s[1], c[1])
        scale_level(2, nc.gpsimd)
        emit_scores(2, 0)
        emit_scores(2, 1)

        # transition 2->3: q-side true cos via DVE; k-side ct = 1 - cos via
        # one ACT Square (sign folded into colS_3); s_3 full width on DVE
        nc.vector.tensor_mul(sq2q, qsl(s[2]), qsl(s[2]))
        nc.vector.tensor_scalar(qsl(c[3]), sq2q, -32.0, 1.0, ALU.mult, ALU.add)
        nc.scalar.activation(ct3k, ksl(s[2]), ACTF.Square,
                             scale=math.sqrt(32.0))
        # switch ACT tables to the exp set (Square/Copy stay available)
        nc.scalar.activation(warm, warm, ACTF.Exp)
        nc.vector.tensor_mul(s[3], s[2], c[2])

        # level 3 scales: separate signs for S/C halves
        for hc in range(2):
            qs = slice(hc * NQ, (hc + 1) * NQ)
            nc.vector.tensor_scalar(SCb[3][:, 0, qs], T[3][:, 0, qs],
                                    colAP(hc, 3), None, ALU.mult)
            nc.vector.tensor_scalar(SCb[3][:, 1, qs], T[3][:, 1, qs],
                                    colAP(hc, 4), None, ALU.mult)
        emit_scores(3, 0, last=True)
        emit_scores(3, 1, last=True)

        # ---- softmax + AV per q-tile
        o_sb = []
        for qt in range(2):
            scp = sc_ps[qt]
            negmax = sm.tile([128, 1], FP32, tag=f"nm{qt}", name=f"nm{qt}")
            nc.vector.reduce_max(negmax, scp, axis=AX.X, negate=True)
            p_sb = sm.tile([128, KP], FP16, tag=f"p{qt}", name=f"p{qt}")
            ssum = sm.tile([128, 1], FP32, tag=f"ss{qt}", name=f"ss{qt}")
            nc.scalar.activation(p_sb, scp, ACTF.Exp, bias=negmax,
                                 accum_out=ssum)
            rs = sm.tile([128, 1], FP32, tag=f"rs{qt}", name=f"rs{qt}")
            nc.vector.reciprocal(rs, ssum)

            pt = ps_pt.tile([128, NK * 128], FP16, tag="pt", name=f"pt{qt}")
            for kc in range(NK):
                nc.tensor.transpose(pt[:, 128 * kc: 128 * (kc + 1)],
                                    p_sb[:, 128 * kc: 128 * (kc + 1)], ident)
            pts = sm.tile([128, NK * 128], FP16, tag=f"pts{qt}", name=f"pts{qt}")
            nc.scalar.activation(pts, pt, ACTF.Copy)
            av = ps_pr.tile([128, V], FP32, tag=f"qp{qt}", name=f"av{qt}")
            for kc in range(NK):
                nc.tensor.matmul(av, pts[:, 128 * kc: 128 * (kc + 1)], v_sb[kc],
                                 start=(kc == 0), stop=(kc == NK - 1))
            o = sm.tile([128, V], FP32, tag=f"o{qt}", name=f"o{qt}")
            nc.scalar.activation(o, av, ACTF.Copy, scale=rs)
            o_sb.append(o)
            nc.sync.dma_start(out=out_d[128 * qt: 128 * (qt + 1), :], in_=o)

    nc.compile()
    return nc


class TileCtx:
    """TileContext + ExitStack in one `with`."""

    def __init__(self, nc):
        self.nc = nc

    def __enter__(self):
        self.ctx = ExitStack()
        self.tc = self.ctx.enter_context(tile.TileContext(self.nc))
        return self.tc, self.ctx

    def __exit__(self, *exc):
        return self.ctx.__exit__(*exc)


def prepare(inputs):
    """Host prep: shards, fit, per-core input maps."""
    queries = np.ascontiguousarray(np.asarray(inputs["queries"], np.float32))
    keys = np.ascontiguousarray(np.asarray(inputs["keys"], np.float32))
    values = np.ascontiguousarray(np.asarray(inputs["values"], np.float32))
    vls = np.asarray(inputs["valid_lens"]).astype(np.int64)
    Wq = np.asarray(inputs["W_q"], np.float32)
    Wk = np.asarray(inputs["W_k"], np.float32)
    wv = np.asarray(inputs["w_v"], np.float32)

    # device projections run on fp16-rounded inputs; match that for ranges
    q16 = queries.astype(np.float16).astype(np.float32)
    k16 = keys.astype(np.float16).astype(np.float32)
    Wq16 = Wq.astype(np.float16).astype(np.float32)
    Wk16 = Wk.astype(np.float16).astype(np.float32)
    qp = [(Wq16 @ q16[b].T).astype(np.float32) for b in range(B)]   # [h, q]
    kp = [(Wk16 @ k16[b].T).astype(np.float32) for b in range(B)]   # [h, k]
    w0, bco = fit_series(qp, kp, vls)
    KP = 128 * max(1, int(math.ceil(vls.max() / 128.0)))

    # scale columns: per hc, [col_0, col_1, col_2, colS_3, colC_3]
    NCOL = 2 * (NLEV + 1)
    cols = np.zeros((128, NCOL), np.float32)
    for hc in range(2):
        wvh = wv[128 * hc: 128 * (hc + 1)]
        base = hc * (NLEV + 1)
        for l in range(3):
            cols[:, base + l] = wvh * bco[l] * (2.0 ** l)
        cols[:, base + 3] = -wvh * bco[3] * 8.0     # Sqb_3 (pairs with ct3k)
        cols[:, base + 4] = wvh * bco[3] * 8.0      # Cqb_3 (pairs with s3k)

    OFF, PX, _cuts = pack_layout(KP)
    NK = KP // 128
    in_maps = []
    for core in range(NCORES):
        b, qlo = core // 2, (core % 2) * NQ
        n = int(vls[b])
        pk = np.zeros((128, PX), np.float16)
        qTm = queries[b, qlo: qlo + NQ].T.astype(np.float16)        # (D, NQ)
        kTm = np.zeros((D, KP), np.float16)
        kTm[:, :n] = keys[b, :n].T.astype(np.float16)
        for i in range(2):
            pk[:, OFF[f"qT{i}"]: OFF[f"qT{i}"] + NQ] = qTm[128 * i: 128 * (i + 1)]
            pk[:, OFF[f"kT{i}"]: OFF[f"kT{i}"] + KP] = kTm[128 * i: 128 * (i + 1)]
            pk[:, OFF[f"wq{i}"]: OFF[f"wq{i}"] + H] = Wq.T[128 * i: 128 * (i + 1)].astype(np.float16)
            pk[:, OFF[f"wk{i}"]: OFF[f"wk{i}"] + H] = Wk.T[128 * i: 128 * (i + 1)].astype(np.float16)
        vm = np.zeros((KP, V), np.float16)
        vm[:n] = values[b, :n].astype(np.float16)
        for i in range(NK):
            pk[:, OFF[f"v{i}"]: OFF[f"v{i}"] + V] = vm[128 * i: 128 * (i + 1)]
        pk[:, OFF["ident"]: OFF["ident"] + 128] = np.eye(128, dtype=np.float16)
        mov = np.zeros((1, KP + 128), np.float16)
        mov[0, :KP] = np.where(np.arange(KP) < n, 0.0, NEGM).astype(np.float16)
        mov[0, KP:] = 1.0
        in_maps.append({"pack": pk, "mo": mov, "cols": cols})
    return w0, bco, KP, in_maps


def kernel(**inputs):
    w0, bco, KP, in_maps = prepare(inputs)
    nc = build_nc(w0, bco, KP)
    res = run_bass_kernel_spmd(nc, in_maps, core_ids=list(range(NCORES)))
    out = np.zeros((B, Q, V), np.float32)
    for core in range(NCORES):
        b, qlo = core // 2, (core % 2) * NQ
        out[b, qlo: qlo + NQ] = res.results[core]["out"]
    return out


# revision 10
# speedup vs baseline: 1.8998x; 1.8998x over previous
"""AdditiveAttention on 8 TRN2 NeuronCores — harmonic-doubling edition.

Math: out = softmax_k(mask(sum_h w_v[h] * tanh(qp[b,q,h] + kp[b,k,h]))) @ values
with qp = queries @ W_q^T, kp = keys @ W_k^T, mask from valid_lens (B,).

tanh(u) ~= sum_{l=0..3} b_l sin(2^l * w0 * u): four harmonics in a pure
power-of-two ladder, so every level comes from the previous by one doubling:
    s[l+1] = s[l]*c[l]   (stored scaled by 1/2^(l+1))
    c[l+1] = 1 - 2*4^l*s[l]^2
No odd-harmonic Chebyshev chain at all. sin(2^l w0 (q+k)) factorizes by angle
addition into PE matmuls contracting over (h, level, trig).

Engine split: ACT does base sin/cos (args kept in [-pi,pi] via w0 choice) plus
softmax exp; DVE does the doubling products (tensor_tensor, 2x fp16) and the
b_l*w_v scale columns (tensor_scalar with per-partition AP scalars, 4x);
Pool (gpsimd) issues one input DMA and takes non-critical scale work; PE does
projections, score matmuls, transposes, AV.

Last-level trick: the k-side cos(8 w0 x) enters only as a matmul operand, so
it is replaced by ct = 1 - cos = 2 sin^2(4 w0 x), computed in ONE ACT Square
op straight from level-2 sin; the dropped constant is a per-query-row score
offset, invisible to softmax. The sign flip folds into the host-built scale
column.

Masking: keys truncated/padded to KP (multiple of 128) >= max(valid_lens); a
rank-1 matmul row adds -60000 to padded score columns so exp underflows to 0.

Sharding: core c handles batch c//2, query rows (c%2)*256..+256.
w0 and b_l are fit host-side from the actual inputs at call time.
"""

import math
from contextlib import ExitStack

import numpy as np

import concourse.bass as bass
import concourse.mybir as mybir
import concourse.tile as tile
from concourse import bacc
from concourse.bass_utils import run_bass_kernel_spmd

B, Q, K, D, H, V = 4, 512, 512, 256, 256, 256
NCORES = 8
NQ = (B * Q) // NCORES          # 256 query rows per core
NLEV = 4                        # harmonics 2^l * w0, l = 0..3
NEGM = -60000.0
FP32 = mybir.dt.float32
FP16 = mybir.dt.float16
AX = mybir.AxisListType
ALU = mybir.AluOpType
ACTF = mybir.ActivationFunctionType


def fit_series(qp, kp, vls):
    """Range analysis + weighted least-squares fit of tanh on the power-of-2
    harmonic ladder. qp/kp: [b][h, *]."""
    umax, xmax = 0.0, 0.0
    for b in range(B):
        kv = kp[b][:, : vls[b]]
        umax = max(umax, (qp[b].max(1) + kv.max(1)).max(),
                   -(qp[b].min(1) + kv.min(1)).min())
        xmax = max(xmax, np.abs(qp[b]).max(), np.abs(kv).max())
    P = max(2.0 * (umax + 0.15), 4.0 * xmax + 0.08)
    w0 = 2.0 * np.pi / P
    u = np.linspace(-(umax + 0.05), umax + 0.05, 4001)
    A = np.stack([np.sin((2.0 ** l) * w0 * u) for l in range(NLEV)], 1)
    wgt = np.exp(-(u ** 2) / (2 * 1.4 ** 2)) + 1e-3
    sw = np.sqrt(wgt)[:, None]
    bco, *_ = np.linalg.lstsq(A * sw, np.tanh(u) * sw[:, 0], rcond=None)
    return float(w0), bco.astype(np.float64)


def pack_layout(KP):
    """Column offsets inside the packed (128, PX) fp16 input tensor. Order
    matters: wk|kT first (k-side spine starts first), wq|qT second, v|ident
    last (needed late)."""
    NK = KP // 128
    names = ([("wk0", H), ("wk1", H), ("kT0", KP), ("kT1", KP),
              ("wq0", H), ("wq1", H), ("qT0", NQ), ("qT1", NQ)]
             + [(f"v{i}", V) for i in range(NK)] + [("ident", 128)])
    off, x = {}, 0
    for nm, w in names:
        off[nm] = x
        x += w
    cutA = off["wq0"]            # end of k-side chunk
    cutB = off["v0"]             # end of q-side chunk
    return off, x, (cutA, cutB)


def build_nc(w0, bco, KP):
    NK = KP // 128
    QW = 2 * NQ                  # q-side width (2 h-chunks)
    KW = 2 * KP                  # k-side width
    CW = QW + KW
    OFF, PX, (CUTA, CUTB) = pack_layout(KP)
    HPI = math.pi / 2
    NCOL = 2 * (NLEV + 1)        # per hc: col_0..2, colS_3, colC_3

    nc = bacc.Bacc()
    pack = nc.declare_dram_parameter("pack", [128, PX], FP16, isOutput=False)
    mo = nc.declare_dram_parameter("mo", [1, KP + 128], FP16, isOutput=False)
    cols = nc.declare_dram_parameter("cols", [128, NCOL], FP32, isOutput=False)
    out_d = nc.declare_dram_parameter("out", [NQ, V], FP32, isOutput=True)

    with TileCtx(nc) as (tc, ctx):
        inp = ctx.enter_context(tc.tile_pool(name="inp", bufs=1))
        harm = ctx.enter_context(tc.tile_pool(name="harm", bufs=1))
        qbp = ctx.enter_context(tc.tile_pool(name="qb", bufs=1))
        sm = ctx.enter_context(tc.tile_pool(name="sm", bufs=1))
        ps_pr = ctx.enter_context(tc.tile_pool(name="psP", bufs=1, space="PSUM"))
        ps_sc = ctx.enter_context(tc.tile_pool(name="psS", bufs=1, space="PSUM"))
        ps_pt = ctx.enter_context(tc.tile_pool(name="psT", bufs=1, space="PSUM"))

        # ---- tiny host-free init on Pool: warmup tile + pi/2 bias column
        warm = inp.tile([1, 128], FP16, tag="warm", name="warm")
        nc.gpsimd.memset(warm, 0.25)
        hpi = inp.tile([128, 1], FP32, tag="hpi", name="hpi")
        nc.gpsimd.memset(hpi, HPI)

        # ---- input DMAs on three parallel queues (HWDGE: SP/ACT; SWDGE: Pool)
        big = inp.tile([128, PX], FP16, tag="big", name="big")
        nc.sync.dma_start(out=big[:, :CUTA], in_=pack[:, :CUTA])         # wk|kT
        nc.scalar.dma_start(out=big[:, CUTA:CUTB], in_=pack[:, CUTA:CUTB])  # wq|qT
        mo_sb = inp.tile([1, KP + 128], FP16, tag="mo", name="mo_sb")
        cols_sb = inp.tile([128, NCOL], FP32, tag="cols", name="cols_sb")
        nc.gpsimd.dma_start(out=mo_sb, in_=mo[:, :])
        nc.gpsimd.dma_start(out=cols_sb, in_=cols[:, :])
        nc.gpsimd.dma_start(out=big[:, CUTB:], in_=pack[:, CUTB:])       # v|ident

        # ACT: trigger the Sin table load ASAP via a warmup op
        nc.scalar.activation(warm, warm, ACTF.Sin, scale=0.001)

        wk_sb = [big[:, OFF[f"wk{i}"]: OFF[f"wk{i}"] + H] for i in range(2)]
        kT_sb = [big[:, OFF[f"kT{i}"]: OFF[f"kT{i}"] + KP] for i in range(2)]
        wq_sb = [big[:, OFF[f"wq{i}"]: OFF[f"wq{i}"] + H] for i in range(2)]
        qT_sb = [big[:, OFF[f"qT{i}"]: OFF[f"qT{i}"] + NQ] for i in range(2)]
        v_sb = [big[:, OFF[f"v{i}"]: OFF[f"v{i}"] + V] for i in range(NK)]
        mrow_sb = mo_sb[:, :KP]
        ones_r = mo_sb[:, KP: KP + 128]
        ident = big[:, OFF["ident"]: OFF["ident"] + 128]

        def colAP(hc, j):
            return cols_sb[:, hc * (NLEV + 1) + j: hc * (NLEV + 1) + j + 1]

        # ---- projections: kp first (k spine), then qp
        kp_ps = [ps_pr.tile([128, KP], FP32, tag=f"kp{hc}", name=f"kp{hc}")
                 for hc in range(2)]
        for hc in range(2):
            for dc in range(2):
                nc.tensor.matmul(kp_ps[hc], wk_sb[dc][:, 128 * hc: 128 * (hc + 1)],
                                 kT_sb[dc], start=(dc == 0), stop=(dc == 1))
        qp_ps = [ps_pr.tile([128, NQ], FP32, tag=f"qp{hc}", name=f"qp{hc}")
                 for hc in range(2)]
        for hc in range(2):
            for dc in range(2):
                nc.tensor.matmul(qp_ps[hc], wq_sb[dc][:, 128 * hc: 128 * (hc + 1)],
                                 qT_sb[dc], start=(dc == 0), stop=(dc == 1))

        # ---- masks open the score accumulation groups early
        sc_ps = [ps_sc.tile([128, KP], FP32, tag=f"sc{qt}", name=f"sc{qt}")
                 for qt in range(2)]
        for qt in range(2):
            nc.tensor.matmul(sc_ps[qt], ones_r, mrow_sb, start=True, stop=False)

        # ---- harmonic tiles: T[l] = [s | c] over [q-side | k-side]
        # layout [128, 2, CW]: [:, 0] = s (stored sin/2^l), [:, 1] = c
        T = [harm.tile([128, 2, CW], FP16, tag=f"T{l}", name=f"T{l}")
             for l in range(NLEV)]
        s = [T[l][:, 0] for l in range(NLEV)]
        c = [T[l][:, 1] for l in range(NLEV)]
        sq = [harm.tile([128, CW], FP16, tag=f"sq{l}", name=f"sq{l}")
              for l in range(2)]
        sq2q = harm.tile([128, QW], FP16, tag="sq2q", name="sq2q")
        ct3k = harm.tile([128, KW], FP16, tag="ct3k", name="ct3k")

        def qsl(ap):                 # q-side slice of a [128, CW] view
            return ap[:, :QW]

        def ksl(ap):                 # k-side slice
            return ap[:, QW:]

        # base level 0: k-side per hc (earliest start), then q-side
        for hc in range(2):
            sl = slice(QW + hc * KP, QW + (hc + 1) * KP)
            nc.scalar.activation(s[0][:, sl], kp_ps[hc], ACTF.Sin, scale=w0)
        for hc in range(2):
            sl = slice(QW + hc * KP, QW + (hc + 1) * KP)
            nc.scalar.activation(c[0][:, sl], kp_ps[hc], ACTF.Sin, scale=w0,
                                 bias=hpi)
        for hc in range(2):
            sl = slice(hc * NQ, (hc + 1) * NQ)
            nc.scalar.activation(s[0][:, sl], qp_ps[hc], ACTF.Sin, scale=w0)
        for hc in range(2):
            sl = slice(hc * NQ, (hc + 1) * NQ)
            nc.scalar.activation(c[0][:, sl], qp_ps[hc], ACTF.Sin, scale=w0,
                                 bias=hpi)

        # ---- scaled q-side stationaries SCb[l] = [Sqb | Cqb], [128, 2, QW]
        SCb = [qbp.tile([128, 2, QW], FP16, tag=f"SCb{l}", name=f"SCb{l}")
               for l in range(NLEV)]

        def scale_level(l, eng):
            """SCb[l] = col_l * [s_l | c_l]|q in one tensor_scalar per hc pair
            of columns (single col covers both trig; see module docstring)."""
            for hc in range(2):
                qs = slice(hc * NQ, (hc + 1) * NQ)
                eng.tensor_scalar(SCb[l][:, :, qs], T[l][:, :, qs],
                                  colAP(hc, l), None, ALU.mult)

        # ---- doubling chain, interleaved with scales and score matmuls.
        # transitions l -> l+1:  sq_l = s_l^2 ;  c_{l+1} = 1 - 2*4^l*sq_l ;
        #                        s_{l+1} = s_l * c_l
        score_mm = []                # collect (l, qt, stationary, moving) order

        def emit_scores(l, qt, last=False):
            """8 matmuls accumulate level l into sc_ps[qt]."""
            for hc in range(2):
                q128 = slice(hc * NQ + qt * 128, hc * NQ + (qt + 1) * 128)
                k_sl = slice(QW + hc * KP, QW + (hc + 1) * KP)
                if l < 3:
                    mvS, mvC = s[l][:, k_sl], c[l][:, k_sl]
                else:
                    mvS, mvC = s[3][:, k_sl], ct3k[:, hc * KP:(hc + 1) * KP]
                fin = last and hc == 1
                nc.tensor.matmul(sc_ps[qt], SCb[l][:, 0, q128], mvC,
                                 start=False, stop=False)
                nc.tensor.matmul(sc_ps[qt], SCb[l][:, 1, q128], mvS,
                                 start=False, stop=fin)

        # level 0 scale + scores
        scale_level(0, nc.vector)
        emit_scores(0, 0)
        emit_scores(0, 1)

        # transition 0->1
        nc.vector.tensor_mul(sq[0], s[0], s[0])
        nc.vector.tensor_scalar(c[1], sq[0], -2.0, 1.0, ALU.mult, ALU.add)
        nc.vector.tensor_mul(s[1], s[0], c[0])
        scale_level(1, nc.vector)
        emit_scores(1, 0)
        emit_scores(1, 1)

        # transition 1->2
        nc.vector.tensor_mul(sq[1], s[1], s[1])
        nc.vector.tensor_scalar(c[2], sq[1], -8.0, 1.0, ALU.mult, ALU.add)
        nc.vector.tensor_mul(s[2], s[1], c[1])
        scale_level(2, nc.vector)
        emit_scores(2, 0)
        emit_scores(2, 1)

        # transition 2->3: q-side true cos via DVE; k-side ct = 1 - cos via
        # one ACT Square (sign folded into colS_3); s_3 full width on DVE
        nc.vector.tensor_mul(sq2q, qsl(s[2]), qsl(s[2]))
        nc.vector.tensor_scalar(qsl(c[3]), sq2q, -32.0, 1.0, ALU.mult, ALU.add)
        nc.scalar.activation(ct3k, ksl(s[2]), ACTF.Square,
                             scale=math.sqrt(32.0))
        # switch ACT tables to the exp set (Square/Copy stay available).
        # warm2 READS c[0] so the scheduler cannot hoist this above the base
        # Sin ops (that would thrash the table back and forth).
        warm2 = inp.tile([128, 128], FP16, tag="warm2", name="warm2")
        nc.scalar.activation(warm2, c[0][:, QW - 128: QW], ACTF.Exp)
        nc.vector.tensor_mul(s[3], s[2], c[2])

        # level 3 scales: separate signs for S/C halves
        for hc in range(2):
            qs = slice(hc * NQ, (hc + 1) * NQ)
            nc.vector.tensor_scalar(SCb[3][:, 0, qs], T[3][:, 0, qs],
                                    colAP(hc, 3), None, ALU.mult)
            nc.vector.tensor_scalar(SCb[3][:, 1, qs], T[3][:, 1, qs],
                                    colAP(hc, 4), None, ALU.mult)
        emit_scores(3, 0, last=True)
        emit_scores(3, 1, last=True)

        # ---- softmax + AV per q-tile
        o_sb = []
        for qt in range(2):
            scp = sc_ps[qt]
            negmax = sm.tile([128, 1], FP32, tag=f"nm{qt}", name=f"nm{qt}")
            nc.vector.reduce_max(negmax, scp, axis=AX.X, negate=True)
            p_sb = sm.tile([128, KP], FP16, tag=f"p{qt}", name=f"p{qt}")
            ssum = sm.tile([128, 1], FP32, tag=f"ss{qt}", name=f"ss{qt}")
            nc.scalar.activation(p_sb, scp, ACTF.Exp, bias=negmax,
                                 accum_out=ssum)
            rs = sm.tile([128, 1], FP32, tag=f"rs{qt}", name=f"rs{qt}")
            nc.vector.reciprocal(rs, ssum)

            pt = ps_pt.tile([128, NK * 128], FP16, tag="pt", name=f"pt{qt}")
            for kc in range(NK):
                nc.tensor.transpose(pt[:, 128 * kc: 128 * (kc + 1)],
                                    p_sb[:, 128 * kc: 128 * (kc + 1)], ident)
            pts = sm.tile([128, NK * 128], FP16, tag=f"pts{qt}", name=f"pts{qt}")
            nc.scalar.activation(pts, pt, ACTF.Copy)
            av = ps_pr.tile([128, V], FP32, tag=f"qp{qt}", name=f"av{qt}")
            for kc in range(NK):
                nc.tensor.matmul(av, pts[:, 128 * kc: 128 * (kc + 1)], v_sb[kc],
                                 start=(kc == 0), stop=(kc == NK - 1))
            o = sm.tile([128, V], FP32, tag=f"o{qt}", name=f"o{qt}")
            nc.scalar.activation(o, av, ACTF.Copy, scale=rs)
            o_sb.append(o)
            nc.sync.dma_start(out=out_d[128 * qt: 128 * (qt + 1), :], in_=o)

    nc.compile()
    return nc


class TileCtx:
    """TileContext + ExitStack in one `with`."""

    def __init__(self, nc):
        self.nc = nc

    def __enter__(self):
        self.ctx = ExitStack()
        self.tc = self.ctx.enter_context(tile.TileContext(self.nc))
        return self.tc, self.ctx

    def __exit__(self, *exc):
        return self.ctx.__exit__(*exc)


def prepare(inputs):
    """Host prep: shards, fit, per-core input maps."""
    queries = np.ascontiguousarray(np.asarray(inputs["queries"], np.float32))
    keys = np.ascontiguousarray(np.asarray(inputs["keys"], np.float32))
    values = np.ascontiguousarray(np.asarray(inputs["values"], np.float32))
    vls = np.asarray(inputs["valid_lens"]).astype(np.int64)
    Wq = np.asarray(inputs["W_q"], np.float32)
    Wk = np.asarray(inputs["W_k"], np.float32)
    wv = np.asarray(inputs["w_v"], np.float32)

    # device projections run on fp16-rounded inputs; match that for ranges
    q16 = queries.astype(np.float16).astype(np.float32)
    k16 = keys.astype(np.float16).astype(np.float32)
    Wq16 = Wq.astype(np.float16).astype(np.float32)
    Wk16 = Wk.astype(np.float16).astype(np.float32)
    qp = [(Wq16 @ q16[b].T).astype(np.float32) for b in range(B)]   # [h, q]
    kp = [(Wk16 @ k16[b].T).astype(np.float32) for b in range(B)]   # [h, k]
    w0, bco = fit_series(qp, kp, vls)
    KP = 128 * max(1, int(math.ceil(vls.max() / 128.0)))

    # scale columns: per hc, [col_0, col_1, col_2, colS_3, colC_3]
    NCOL = 2 * (NLEV + 1)
    cols = np.zeros((128, NCOL), np.float32)
    for hc in range(2):
        wvh = wv[128 * hc: 128 * (hc + 1)]
        base = hc * (NLEV + 1)
        for l in range(3):
            cols[:, base + l] = wvh * bco[l] * (2.0 ** l)
        cols[:, base + 3] = -wvh * bco[3] * 8.0     # Sqb_3 (pairs with ct3k)
        cols[:, base + 4] = wvh * bco[3] * 8.0      # Cqb_3 (pairs with s3k)

    OFF, PX, _cuts = pack_layout(KP)
    NK = KP // 128
    in_maps = []
    for core in range(NCORES):
        b, qlo = core // 2, (core % 2) * NQ
        n = int(vls[b])
        pk = np.zeros((128, PX), np.float16)
        qTm = queries[b, qlo: qlo + NQ].T.astype(np.float16)        # (D, NQ)
        kTm = np.zeros((D, KP), np.float16)
        kTm[:, :n] = keys[b, :n].T.astype(np.float16)
        for i in range(2):
            pk[:, OFF[f"qT{i}"]: OFF[f"qT{i}"] + NQ] = qTm[128 * i: 128 * (i + 1)]
            pk[:, OFF[f"kT{i}"]: OFF[f"kT{i}"] + KP] = kTm[128 * i: 128 * (i + 1)]
            pk[:, OFF[f"wq{i}"]: OFF[f"wq{i}"] + H] = Wq.T[128 * i: 128 * (i + 1)].astype(np.float16)
            pk[:, OFF[f"wk{i}"]: OFF[f"wk{i}"] + H] = Wk.T[128 * i: 128 * (i + 1)].astype(np.float16)
        vm = np.zeros((KP, V), np.float16)
        vm[:n] = values[b, :n].astype(np.float16)
        for i in range(NK):
            pk[:, OFF[f"v{i}"]: OFF[f"v{i}"] + V] = vm[128 * i: 128 * (i + 1)]
        pk[:, OFF["ident"]: OFF["ident"] + 128] = np.eye(128, dtype=np.float16)
        mov = np.zeros((1, KP + 128), np.float16)
        mov[0, :KP] = np.where(np.arange(KP) < n, 0.0, NEGM).astype(np.float16)
        mov[0, KP:] = 1.0
        in_maps.append({"pack": pk, "mo": mov, "cols": cols})
    return w0, bco, KP, in_maps


def kernel(**inputs):
    w0, bco, KP, in_maps = prepare(inputs)
    nc = build_nc(w0, bco, KP)
    res = run_bass_kernel_spmd(nc, in_maps, core_ids=list(range(NCORES)))
    out = np.zeros((B, Q, V), np.float32)
    for core in range(NCORES):
        b, qlo = core // 2, (core % 2) * NQ
        out[b, qlo: qlo + NQ] = res.results[core]["out"]
    return out
